# revision 49
# baseline (speedup 1.0000x reference)
"""Multi-head attention (B=2, T=2048, D=1024, 16 heads) on 8 TRN2 NeuronCores.

Sharding: DP2 x TP4 — core c handles batch c//4 and 4 heads (c%4).
Per core: QKV projections [2048 tok, 256 dv] in bf16, causal flash-style
attention in the S^T = K @ Q^T form, partial output projection
o_c = attn_out_c @ Wo[:, cols_c].T written as bf16; host sums the 4
partials per batch (tensor-parallel all-reduce on host).

Key engine choreography (vs the f32r baseline at 293us):
- everything bf16 on the PE: narrow matmuls run at 1 cycle/row at any
  width, so diagonal-block scores/AV/exp shrink to the causal live range.
- causal mask applied POST-exp as a multiplicative [128,128] pattern on
  the idle-ish Vector engine, off the Scalar(exp) critical path.
- V^T produced directly by the projection (stationary = xt tile,
  moving = Wv^T slice): no transposes at all, one cast per token-pair.
- PSUM evacuations on Vector; exp on Scalar; Vector also does the small
  normalization ops and the post-exp causal mask multiplies.
- o-proj + next-chunk projections queued as fine-grained deferred items
  popped between attention pairs: the PE never drains, stays at max
  pstate (2.4 GHz), and weight loads hide under streams.
- PSUM: sps 2x2 banks + av 2 banks + shared proj/o-proj ring 2 banks = 8.
"""

import sys

sys.path.insert(0, "/opt/trn_rl_repo")

import numpy as np

B, T, D = 2, 2048, 1024
NCORES = 8
DV = 256          # head dims per core (4 heads x 64)
DH = 64
NHPC = 4          # heads per core
CH = 512          # tq chunk width
NCH_B = T // CH   # 4 chunks per batch
TK = 128          # tk tile
NTK = T // TK     # 16 tiles per batch
ND = D // 128     # 8 contraction tiles
DVA = DH + 1      # V columns incl ones
NU = NTK * NHPC   # 64 v-blocks per core

_cache = {}


def _plan_blocks(mask):
    """Classify (tq-chunk, tk-tile) blocks of the [T, T] keep-mask.

    Returns (plans, patterns): plans[jj] = ordered visit list of
    (i, l0, l1, m0, m1, pat); patterns = list of [128, 512] float mask
    tiles (1.0 keep / 0.0 drop), hull content left-aligned.
    """
    patterns = []
    pattern_idx = {}
    plans = []
    for jj in range(NCH_B):
        vis = []
        for i in range(NTK):
            blk = mask[jj * CH:(jj + 1) * CH, i * TK:(i + 1) * TK].T  # [tk, tq]
            cola = blk.any(axis=0)
            if not cola.any():
                continue
            colk = blk.all(axis=0)
            l0 = int(np.argmax(cola))
            l1 = int(len(cola) - np.argmax(cola[::-1]))
            mcols = (cola & ~colk) | (~cola & (np.arange(CH) >= l0)
                                      & (np.arange(CH) < l1))
            if mcols.any():
                m0 = int(np.argmax(mcols))
                m1 = int(len(mcols) - np.argmax(mcols[::-1]))
                key = blk[:, m0:m1].tobytes()
                if key not in pattern_idx:
                    pattern_idx[key] = len(patterns)
                    pat = np.zeros((TK, CH), np.float32)
                    pat[:, 0:m1 - m0] = blk[:, m0:m1]
                    patterns.append(pat)
                vis.append([i, l0, l1, m0, m1, pattern_idx[key]])
            else:
                vis.append([i, l0, l1, 0, 0, -1])
        # widest first (PSUM-start nesting); among equal widths, masked
        # (diagonal) visits first so their mask-mult latency hides under
        # the unmasked full tiles that follow
        vis.sort(key=lambda v: (-(v[2] - v[1]), 0 if v[5] >= 0 else 1))
        ok = True
        if vis:
            c0, c1 = vis[0][1], vis[0][2]
            for v in vis[1:]:
                if v[1] < c0 or v[2] > c1:
                    ok = False
        if not ok:
            # general fallback: full-width everything, full masks
            vis = []
            for i in range(NTK):
                blk = mask[jj * CH:(jj + 1) * CH, i * TK:(i + 1) * TK].T
                if not blk.any():
                    continue
                if blk.all():
                    vis.append([i, 0, CH, 0, 0, -1])
                    continue
                key = blk.tobytes()
                if key not in pattern_idx:
                    pattern_idx[key] = len(patterns)
                    patterns.append(blk.astype(np.float32))
                vis.append([i, 0, CH, 0, CH, pattern_idx[key]])
        plans.append(vis)
    return plans, patterns


def _build(plan_key, n_pat):
    import concourse.bacc as bacc
    import concourse.mybir as mybir
    import concourse.tile as tile

    F32 = mybir.dt.float32
    BF16 = mybir.dt.bfloat16
    EXP = mybir.ActivationFunctionType.Exp
    MULT = mybir.AluOpType.mult

    plans = []
    idx = 0
    for jj in range(NCH_B):
        nv = plan_key[idx]; idx += 1
        vis = []
        for _ in range(nv):
            vis.append(plan_key[idx:idx + 6]); idx += 6
        plans.append(vis)

    nc = bacc.Bacc("TRN2", target_bir_lowering=False, debug=False,
                   num_devices=NCORES)

    xt_d = nc.dram_tensor("xt", [D, T], BF16, kind="ExternalInput").ap()
    wq_d = nc.dram_tensor("wq", [128, 2 * ND * 128], BF16,
                          kind="ExternalInput").ap()
    wk_d = nc.dram_tensor("wk", [128, 2 * ND * 128], BF16,
                          kind="ExternalInput").ap()
    wv_d = nc.dram_tensor("wv", [128, 2 * ND * 128], BF16,
                          kind="ExternalInput").ap()
    wo_d = nc.dram_tensor("wo", [128, 2 * D], BF16, kind="ExternalInput").ap()
    nmask = max(n_pat, 1)
    mask_d = nc.dram_tensor("mask", [nmask, TK, CH], BF16,
                            kind="ExternalInput").ap()
    o_d = nc.dram_tensor("o", [T, D], BF16, kind="ExternalOutput").ap()

    with tile.TileContext(nc) as tc:
        with tc.tile_pool(name="consts", bufs=1) as consts, \
             tc.tile_pool(name="perm", bufs=1) as perm, \
             tc.tile_pool(name="xt_pool", bufs=16) as xtp, \

             tc.tile_pool(name="p_pool", bufs=5) as ppool, \
             tc.tile_pool(name="rec_pool", bufs=2) as recp, \
             tc.tile_pool(name="osb_pool", bufs=3) as obp, \
             tc.tile_pool(name="dram_pool", bufs=2, space="DRAM") as drp:
            wsb = {}
            for nm in ("q", "k"):
                wsb[nm] = consts.tile([128, 2, ND, 128], BF16,
                                      name=f"w{nm}_sb")
            # V weights in moving layout [dpart, dtile, 256 dv]: V^T comes
            # straight out of the projection (stationary = xt tile), no
            # transposes needed
            wsb["v"] = consts.tile([128, ND, DV], BF16, name="wv_sb")
            wo_sb = consts.tile([128, 2, D], BF16, name="wo_sb")
            mask_sb = consts.tile([128, nmask, CH], BF16, name="mask_sb")

            qT = [perm.tile([128, T], BF16, name=f"qT{h}") for h in (0, 1)]
            kT = [perm.tile([128, T], BF16, name=f"kT{h}") for h in (0, 1)]
            outT = perm.tile([128, 2, T], BF16, name="outT")
            # one V tile per chunk (4 tk-tiles x 4 heads x 65) — a single
            # big tile exceeds the subtile write-tracking budget and makes
            # every AV falsely wait on ALL later-chunk V writes
            vsbs = [perm.tile([128, 4 * NHPC, DVA], BF16, name=f"vsb{c}")
                    for c in range(NCH_B)]
            for c in range(NCH_B):
                nc.gpsimd.memset(vsbs[c][:, :, DH:DVA], 1.0)

            xt_tiles = {}

            def prefetch_xt(jj):
                for d in range(ND):
                    xt = xtp.tile([128, CH], BF16, tag="xt",
                                  name=f"xt{jj}_{d}")
                    nc.sync.dma_start(
                        xt[:], xt_d[d * 128:(d + 1) * 128,
                                    jj * CH:(jj + 1) * CH])
                    xt_tiles[(jj, d)] = xt

            def evac(jj, nm, dvh, acc):
                cs = slice(jj * CH, (jj + 1) * CH)
                if nm == "q":
                    nc.vector.tensor_copy(qT[dvh][:, cs], acc[:])
                else:
                    nc.vector.tensor_copy(kT[dvh][:, cs], acc[:])

            def proj_subblock(jj, nm, dvh):
                def run(pool):
                    acc = pool.tile([128, CH], F32, tag="acc",
                                    name=f"acc_{nm}{jj}_{dvh}")
                    for d in range(ND):
                        nc.tensor.matmul(acc[:], wsb[nm][:, dvh, d, :],
                                         xt_tiles[(jj, d)][:],
                                         start=(d == 0), stop=(d == ND - 1))
                    evac(jj, nm, dvh, acc)
                return run

            def vproj_pair(jj, tp):
                # V^T for token-tiles 2tp, 2tp+1: out psum [128 tok, 2, 256]
                def run(pool):
                    acc = pool.tile([128, CH], F32, tag="acc",
                                    name=f"vacc{jj}_{tp}")
                    accv = acc[:].rearrange("p (a b) -> p a b", b=DV)
                    for tl in (0, 1):
                        t = tp * 2 + tl
                        for d in range(ND):
                            nc.tensor.matmul(
                                accv[:, tl, :],
                                xt_tiles[(jj, d)][:, t * 128:(t + 1) * 128],
                                wsb["v"][:, d, :],
                                start=(d == 0), stop=(d == ND - 1))
                    nc.vector.tensor_copy(
                        vsbs[jj][:, 8 * tp:8 * tp + 8, 0:DH],
                        acc[:].rearrange("p (u c) -> p u c", c=DH))
                return run

            def oproj_half(jj, tt, half, evac_eng="v"):
                def run(pool):
                    o = pool.tile([128, CH], F32, tag="acc",
                                  name=f"o{jj}_{tt}_{half}")
                    ts = jj * CH + tt * 128
                    for hp in (0, 1):
                        nc.tensor.matmul(
                            o[:], outT[:, hp, ts:ts + 128],
                            wo_sb[:, hp, half * CH:(half + 1) * CH],
                            start=(hp == 0), stop=(hp == 1))
                    osb = obp.tile([128, CH], BF16, tag="osb",
                                   name=f"osb{jj}_{tt}_{half}")
                    if evac_eng == "s":
                        nc.scalar.activation(
                            osb[:], o[:], mybir.ActivationFunctionType.Copy)
                    else:
                        nc.vector.tensor_copy(osb[:], o[:])
                    nc.sync.dma_start(
                        o_d[ts:ts + 128, half * CH:(half + 1) * CH], osb[:])
                return run

            deferred = []

            reserve = [0]

            def pop_deferred(pool, n=1):
                for _ in range(n):
                    if len(deferred) > reserve[0]:
                        deferred.pop(0)(pool)

            def normalize(jj, hp, av, fillp, mid_hook=None):
                tq0 = jj * CH
                su0 = recp.tile([1, CH], F32, tag="su0", name=f"su0{jj}_{hp}")
                su1 = recp.tile([1, CH], F32, tag="su1", name=f"su1{jj}_{hp}")
                outu = recp.tile([128, CH], F32, tag="outu",
                                 name=f"outu{jj}_{hp}")
                nc.vector.tensor_copy(su0[:], av[0][DH:DVA, :])
                nc.vector.tensor_copy(su1[:], av[1][DH:DVA, :])
                dr = drp.tile([2, CH], F32, tag="dr", name=f"dr{jj}_{hp}")
                nc.sync.dma_start(dr[0:1, :], su0[:])
                nc.sync.dma_start(dr[1:2, :], su1[:])
                nc.vector.tensor_copy(outu[0:DH, :], av[0][0:DH, :])
                nc.vector.tensor_copy(outu[DH:128, :], av[1][0:DH, :])
                r8 = recp.tile([128, 8], F32, tag="r8", name=f"r8{jj}_{hp}")
                nc.sync.dma_start(
                    r8[:], dr[:].rearrange("a b -> (a b)").rearrange(
                        "(p j) -> p j", j=8))
                r8r = recp.tile([128, 8], F32, tag="r8r", name=f"r8r{jj}_{hp}")
                nc.vector.reciprocal(r8r[:], r8[:])
                dr2 = drp.tile([2, CH], F32, tag="dr2", name=f"dr2{jj}_{hp}")
                nc.sync.dma_start(
                    dr2[:].rearrange("a b -> (a b)").rearrange(
                        "(p j) -> p j", j=8), r8r[:])
                rbc = recp.tile([128, CH], F32, tag="rbc",
                                name=f"rbc{jj}_{hp}")
                nc.sync.dma_start(rbc[0:DH, :],
                                  dr2[0:1, :].broadcast_to([DH, CH]))
                nc.sync.dma_start(rbc[DH:128, :],
                                  dr2[1:2, :].broadcast_to([DH, CH]))
                nc.vector.tensor_tensor(
                    out=outT[:, hp, tq0:tq0 + CH], in0=outu[:],
                    in1=rbc[:], op=MULT)
                pop_deferred(fillp)
                if mid_hook is not None:
                    mid_hook()

            def attention_chunk(jj, spsp, avp, fillp, mid_hook=None):
                vis = plans[jj]
                if not vis:
                    return
                tq0 = jj * CH
                first_i = vis[0][0]
                last_i = vis[-1][0]
                avs = {}

                hooked = [mid_hook]

                def emit_av(items):
                    done_hps = []
                    for (hp, (i, l0, l1, m0, m1, pat)), p in items:
                        if hp not in avs:
                            avs[hp] = [avp.tile([DVA, CH], F32, tag="av",
                                                name=f"av{h}_{jj}_{hp}")
                                       for h in (0, 1)]
                        for h in (0, 1):
                            u = (i % 4) * NHPC + hp * 2 + h
                            nc.tensor.matmul(
                                avs[hp][h][:, l0:l1],
                                vsbs[i // 4][:, u, :],
                                p[:, h, l0:l1],
                                start=(i == first_i), stop=(i == last_i),
                                skip_group_check=True)
                        if i == last_i:
                            done_hps.append(hp)
                    for hp in done_hps:
                        normalize(jj, hp, avs[hp], fillp, hooked[0])
                        hooked[0] = None

                stream = [(hp, v) for hp in (0, 1) for v in vis]
                pend = None
                pairs = [stream[x:x + 2] for x in range(0, len(stream), 2)]
                for pidx, pair in enumerate(pairs):
                    cur = []
                    for hp, v in pair:
                        i, l0, l1, m0, m1, pat = v
                        ks = slice(i * TK, (i + 1) * TK)
                        sps = spsp.tile([128, 2, CH], F32, tag="sps",
                                        name=f"sps{jj}_{hp}_{i}")
                        for h in (0, 1):
                            hs = slice(h * DH, (h + 1) * DH)
                            nc.tensor.matmul(
                                sps[:, h, l0:l1], kT[hp][hs, ks],
                                qT[hp][hs, tq0 + l0:tq0 + l1],
                                start=True, stop=True)
                        cur.append(((hp, v), sps))
                    if pidx < len(pairs) - 1:
                        pop_deferred(fillp)
                    cur2 = []
                    for hv, sps in cur:
                        hp, (i, l0, l1, m0, m1, pat) = hv
                        p = ppool.tile([128, 2, CH], BF16, tag="p",
                                       name=f"p{jj}_{hp}_{i}")
                        nc.scalar.activation(p[:, :, l0:l1],
                                             sps[:, :, l0:l1], EXP)
                        if pat >= 0:
                            for h in (0, 1):
                                nc.vector.tensor_tensor(
                                    out=p[:, h, m0:m1], in0=p[:, h, m0:m1],
                                    in1=mask_sb[:, pat, 0:m1 - m0], op=MULT)
                        cur2.append((hv, p))
                    if pidx < len(pairs) - 1:
                        pop_deferred(fillp)
                    if pend is not None:
                        emit_av(pend)
                    pend = cur2
                emit_av(pend)

            # ---------------- emission schedule ----------------
            # DMA order: xt chunk0, dvh0 weights, xt chunk1, dvh1 weights —
            # chunk-0 dvh0 projections start as soon as the first tiles land
            # and attn(0)'s chunk-1 proj fillers get their xt early
            # first-chunk tiles interleaved per d so the first projection
            # matmul can start after ~0.3MB instead of 2.5MB
            for d in range(ND):
                xt = xtp.tile([128, CH], BF16, tag="xt", name=f"xt0_{d}")
                nc.sync.dma_start(xt[:], xt_d[d * 128:(d + 1) * 128, 0:CH])
                xt_tiles[(0, d)] = xt
                for nm, dt_ in (("q", wq_d), ("k", wk_d)):
                    nc.sync.dma_start(wsb[nm][:, 0, d, :],
                                      dt_[:, d * 128:(d + 1) * 128])
            nc.sync.dma_start(
                wsb["v"][:].rearrange("p e c -> p (e c)"), wv_d[:])
            prefetch_xt(1)
            for nm, dt_ in (("q", wq_d), ("k", wk_d)):
                nc.sync.dma_start(
                    wsb[nm][:, 1].rearrange("p e c -> p (e c)"),
                    dt_[:, ND * 128:2 * ND * 128])
            with tc.tile_pool(name="proj0_ps", bufs=1, space="PSUM") as pps0:
                acc0 = {}
                for dvh in (0, 1):
                    for nm in ("q", "k"):
                        acc0[(nm, dvh)] = pps0.tile(
                            [128, CH], F32, tag=f"a{nm}{dvh}",
                            name=f"acc0_{nm}{dvh}")
                for dvh in (0, 1):
                    for d in range(ND):
                        for nm in ("q", "k"):
                            nc.tensor.matmul(
                                acc0[(nm, dvh)][:], wsb[nm][:, dvh, d, :],
                                xt_tiles[(0, d)][:],
                                start=(d == 0), stop=(d == ND - 1))
                    for nm in ("q", "k"):
                        evac(0, nm, dvh, acc0[(nm, dvh)])
                    if dvh == 0:
                        vproj_pair(0, 0)(pps0)
                for tp in (1,):
                    vproj_pair(0, tp)(pps0)
            nc.sync.dma_start(wo_sb[:].rearrange("p a b -> p (a b)"), wo_d[:])
            for mi in range(n_pat):
                nc.sync.dma_start(mask_sb[:, mi, :], mask_d[mi])

            pending_oproj = []
            for jj in range(NCH_B):
                if jj + 1 < NCH_B:
                    for dvh in (0, 1):
                        for nm in ("q", "k"):
                            deferred.append(proj_subblock(jj + 1, nm, dvh))
                    for tp in (0, 1):
                        deferred.append(vproj_pair(jj + 1, tp))
                # o-proj of chunk jj-1: queued here (one full chunk after its
                # normalization started) so popped items never wait on the
                # reciprocal bounce and block the in-order PE queue
                deferred.extend(pending_oproj)
                pending_oproj = []
                # xt prefetch for chunk jj+2 fires mid-chunk (after the
                # first normalize) so this bulk transfer queues BEHIND the
                # V-transpose DMAs of chunk jj+1's fillers on the SP queue
                hook = ((lambda c=jj + 2: prefetch_xt(c))
                        if jj + 2 < NCH_B else None)
                # on the last chunk, hold back a few ready o-proj items so
                # the PE has work during the final normalization bounce
                reserve[0] = 4 if jj == NCH_B - 1 else 0
                with tc.tile_pool(name=f"s_ps{jj}", bufs=2,
                                  space="PSUM") as spsp, \
                     tc.tile_pool(name=f"av_ps{jj}", bufs=3,
                                  space="PSUM") as avp, \
                     tc.tile_pool(name=f"fill_ps{jj}", bufs=1,
                                  space="PSUM") as fillp:
                    attention_chunk(jj, spsp, avp, fillp, hook)
                ev = "s" if jj == NCH_B - 1 else "v"
                pending_oproj = [oproj_half(jj, tt, half, ev)
                                 for tt in range(4) for half in (0, 1)]

            reserve[0] = 0
            with tc.tile_pool(name="tail_ps", bufs=4, space="PSUM") as tailp:
                deferred.extend(pending_oproj)
                while deferred:
                    deferred.pop(0)(tailp)

    nc.compile()
    return nc


def kernel(x, Wq, Wk, Wv, Wo, attn_mask):
    import ml_dtypes
    import concourse.bass_utils as _bu

    BF = ml_dtypes.bfloat16
    x = np.asarray(x, dtype=np.float32)
    Wq = np.asarray(Wq, dtype=np.float32)
    Wk = np.asarray(Wk, dtype=np.float32)
    Wv = np.asarray(Wv, dtype=np.float32)
    Wo = np.asarray(Wo, dtype=np.float32)
    mask = np.asarray(attn_mask).astype(bool)

    plans, patterns = _plan_blocks(mask)
    n_pat = len(patterns)
    key_l = []
    for vis in plans:
        key_l.append(len(vis))
        for v in vis:
            key_l.extend(v)
    key = tuple(key_l)
    if key not in _cache:
        _cache[key] = _build(key, n_pat)
    nc = _cache[key]

    mask_arr = (np.stack(patterns).astype(BF) if n_pat
                else np.zeros((1, TK, CH), BF))
    mask_arr = np.ascontiguousarray(mask_arr)

    xts = [np.ascontiguousarray(x[b].reshape(T, D).T.astype(BF))
           for b in range(B)]

    def wlayout(W, rows, scale=1.0):
        Wc = (W[rows, :] * scale).astype(np.float32)  # [256, 1024]
        # -> [128 dpart, 2 dvh, 8 dtile, 128 dvcol] flat [128, 2048]
        m = Wc.T.reshape(ND, 128, 2, 128).transpose(1, 2, 0, 3)
        return np.ascontiguousarray(m.reshape(128, 2 * ND * 128).astype(BF))

    in_maps = []
    for c in range(NCORES):
        b, g = divmod(c, NCORES // B)
        rows = slice(g * DV, (g + 1) * DV)
        wo_dev = Wo[:, rows].T.reshape(2, 128, D).transpose(1, 0, 2)
        in_maps.append({
            "xt": xts[b],
            "wq": wlayout(Wq, rows, 1.0 / np.sqrt(DH)),
            "wk": wlayout(Wk, rows),
            "wv": np.ascontiguousarray(
                Wv[rows, :].astype(np.float32).T
                .reshape(ND, 128, DV).transpose(1, 0, 2)
                .reshape(128, ND * DV).astype(BF)),
            "wo": np.ascontiguousarray(
                wo_dev.reshape(128, 2 * D).astype(BF)),
            "mask": mask_arr,
        })

    res = _bu.run_bass_kernel_spmd(nc, in_maps, core_ids=list(range(NCORES)))
    out = np.zeros((B, T, D), dtype=np.float32)
    for c in range(NCORES):
        b = c // (NCORES // B)
        out[b] += np.asarray(res.results[c]["o"], dtype=np.float32)
    return out


# revision 55
# speedup vs baseline: 1.0767x; 1.0767x over previous
"""Multi-head attention (B=2, T=2048, D=1024, 16 heads) on 8 TRN2 NeuronCores.

Sharding: DP2 x TP4 — core c handles batch c//4 and 4 heads (c%4).
Per core: QKV projections [2048 tok, 256 dv] in bf16, causal flash-style
attention in the S^T = K @ Q^T form, partial output projection
o_c = attn_out_c @ Wo[:, cols_c].T written as bf16; host sums the 4
partials per batch (tensor-parallel all-reduce on host).

Key engine choreography (vs the f32r baseline at 293us):
- everything bf16 on the PE: narrow matmuls run at 1 cycle/row at any
  width, so diagonal-block scores/AV/exp shrink to the causal live range.
- causal mask applied POST-exp as a multiplicative [128,128] pattern on
  the idle-ish Vector engine, off the Scalar(exp) critical path.
- V^T produced directly by the projection (stationary = xt tile,
  moving = Wv^T slice): no transposes at all, one cast per token-pair.
- PSUM evacuations on Vector; exp on Scalar; Vector also does the small
  normalization ops and the post-exp causal mask multiplies.
- o-proj + next-chunk projections queued as fine-grained deferred items
  popped between attention pairs: the PE never drains, stays at max
  pstate (2.4 GHz), and weight loads hide under streams.
- PSUM: sps 2x2 banks + av 2 banks + shared proj/o-proj ring 2 banks = 8.
"""

import sys

sys.path.insert(0, "/opt/trn_rl_repo")

import numpy as np

B, T, D = 2, 2048, 1024
NCORES = 8
DV = 256          # head dims per core (4 heads x 64)
DH = 64
NHPC = 4          # heads per core
CH = 512          # tq chunk width
NCH_B = T // CH   # 4 chunks per batch
TK = 128          # tk tile
NTK = T // TK     # 16 tiles per batch
ND = D // 128     # 8 contraction tiles
DVA = DH + 1      # V columns incl ones
NU = NTK * NHPC   # 64 v-blocks per core

_cache = {}


def _plan_blocks(mask):
    """Classify (tq-chunk, tk-tile) blocks of the [T, T] keep-mask.

    Returns (plans, patterns): plans[jj] = ordered visit list of
    (i, l0, l1, m0, m1, pat); patterns = list of [128, 512] float mask
    tiles (1.0 keep / 0.0 drop), hull content left-aligned.
    """
    patterns = []
    pattern_idx = {}
    plans = []
    for jj in range(NCH_B):
        vis = []
        for i in range(NTK):
            blk = mask[jj * CH:(jj + 1) * CH, i * TK:(i + 1) * TK].T  # [tk, tq]
            cola = blk.any(axis=0)
            if not cola.any():
                continue
            colk = blk.all(axis=0)
            l0 = int(np.argmax(cola))
            l1 = int(len(cola) - np.argmax(cola[::-1]))
            mcols = (cola & ~colk) | (~cola & (np.arange(CH) >= l0)
                                      & (np.arange(CH) < l1))
            if mcols.any():
                m0 = int(np.argmax(mcols))
                m1 = int(len(mcols) - np.argmax(mcols[::-1]))
                key = blk[:, m0:m1].tobytes()
                if key not in pattern_idx:
                    pattern_idx[key] = len(patterns)
                    pat = np.zeros((TK, CH), np.float32)
                    pat[:, 0:m1 - m0] = blk[:, m0:m1]
                    patterns.append(pat)
                vis.append([i, l0, l1, m0, m1, pattern_idx[key]])
            else:
                vis.append([i, l0, l1, 0, 0, -1])
        # widest first (PSUM-start nesting); among equal widths, masked
        # (diagonal) visits first so their mask-mult latency hides under
        # the unmasked full tiles that follow
        vis.sort(key=lambda v: (-(v[2] - v[1]), 0 if v[5] >= 0 else 1))
        ok = True
        if vis:
            c0, c1 = vis[0][1], vis[0][2]
            for v in vis[1:]:
                if v[1] < c0 or v[2] > c1:
                    ok = False
        if not ok:
            # general fallback: full-width everything, full masks
            vis = []
            for i in range(NTK):
                blk = mask[jj * CH:(jj + 1) * CH, i * TK:(i + 1) * TK].T
                if not blk.any():
                    continue
                if blk.all():
                    vis.append([i, 0, CH, 0, 0, -1])
                    continue
                key = blk.tobytes()
                if key not in pattern_idx:
                    pattern_idx[key] = len(patterns)
                    patterns.append(blk.astype(np.float32))
                vis.append([i, 0, CH, 0, CH, pattern_idx[key]])
        plans.append(vis)
    return plans, patterns


def _build(plan_key, n_pat):
    import concourse.bacc as bacc
    import concourse.mybir as mybir
    import concourse.tile as tile

    F32 = mybir.dt.float32
    BF16 = mybir.dt.bfloat16
    EXP = mybir.ActivationFunctionType.Exp
    MULT = mybir.AluOpType.mult

    plans = []
    idx = 0
    for jj in range(NCH_B):
        nv = plan_key[idx]; idx += 1
        vis = []
        for _ in range(nv):
            vis.append(plan_key[idx:idx + 6]); idx += 6
        plans.append(vis)

    nc = bacc.Bacc("TRN2", target_bir_lowering=False, debug=False,
                   num_devices=NCORES)

    xt_d = nc.dram_tensor("xt", [D, T], BF16, kind="ExternalInput").ap()
    wq_d = nc.dram_tensor("wq", [128, 2 * ND * 128], BF16,
                          kind="ExternalInput").ap()
    wk_d = nc.dram_tensor("wk", [128, 2 * ND * 128], BF16,
                          kind="ExternalInput").ap()
    wv_d = nc.dram_tensor("wv", [128, 2 * ND * 128], BF16,
                          kind="ExternalInput").ap()
    wo_d = nc.dram_tensor("wo", [128, 2 * D], BF16, kind="ExternalInput").ap()
    nmask = max(n_pat, 1)
    mask_d = nc.dram_tensor("mask", [nmask, TK, CH], BF16,
                            kind="ExternalInput").ap()
    o_d = nc.dram_tensor("o", [T, D], BF16, kind="ExternalOutput").ap()

    with tile.TileContext(nc) as tc:
        with tc.tile_pool(name="consts", bufs=1) as consts, \
             tc.tile_pool(name="perm", bufs=1) as perm, \
             tc.tile_pool(name="xt_pool", bufs=3) as xtp, \

             tc.tile_pool(name="p_pool", bufs=5) as ppool, \
             tc.tile_pool(name="rec_pool", bufs=2) as recp, \
             tc.tile_pool(name="osb_pool", bufs=3) as obp, \
             tc.tile_pool(name="dram_pool", bufs=2, space="DRAM") as drp:
            wsb = {}
            for nm in ("q", "k"):
                wsb[nm] = consts.tile([128, 2, ND, 128], BF16,
                                      name=f"w{nm}_sb")
            # V weights in moving layout [dpart, dtile, 256 dv]: V^T comes
            # straight out of the projection (stationary = xt tile), no
            # transposes needed
            wsb["v"] = consts.tile([128, ND, DV], BF16, name="wv_sb")
            wo_sb = consts.tile([128, 2, D], BF16, name="wo_sb")
            mask_sb = consts.tile([128, nmask, CH], BF16, name="mask_sb")

            qT = [perm.tile([128, T], BF16, name=f"qT{h}") for h in (0, 1)]
            kT = [perm.tile([128, T], BF16, name=f"kT{h}") for h in (0, 1)]
            outT = perm.tile([128, 2, T], BF16, name="outT")
            # one V tile per chunk (4 tk-tiles x 4 heads x 65) — a single
            # big tile exceeds the subtile write-tracking budget and makes
            # every AV falsely wait on ALL later-chunk V writes
            vsbs = [perm.tile([128, 4 * NHPC, DVA], BF16, name=f"vsb{c}")
                    for c in range(NCH_B)]
            for c in range(NCH_B):
                nc.gpsimd.memset(vsbs[c][:, :, DH:DVA], 1.0)

            xt_tiles = {}

            def prefetch_xt(jj):
                # one batched DMA per chunk: 8 separate DMAs would cost
                # ~565ns each of serial SP-sequencer issue time
                xt = xtp.tile([128, ND, CH], BF16, tag="xt", name=f"xt{jj}")
                nc.sync.dma_start(
                    xt[:], xt_d[:, jj * CH:(jj + 1) * CH].rearrange(
                        "(e p) c -> p e c", p=128))
                xt_tiles[jj] = xt

            def evac(jj, nm, dvh, acc):
                cs = slice(jj * CH, (jj + 1) * CH)
                if nm == "q":
                    nc.vector.tensor_copy(qT[dvh][:, cs], acc[:])
                else:
                    nc.vector.tensor_copy(kT[dvh][:, cs], acc[:])

            def proj_subblock(jj, nm, dvh):
                def run(pool):
                    acc = pool.tile([128, CH], F32, tag="acc",
                                    name=f"acc_{nm}{jj}_{dvh}")
                    for d in range(ND):
                        nc.tensor.matmul(acc[:], wsb[nm][:, dvh, d, :],
                                         xt_tiles[jj][:, d, :],
                                         start=(d == 0), stop=(d == ND - 1))
                    evac(jj, nm, dvh, acc)
                return run

            def vproj_pair(jj, tp):
                # V^T for token-tiles 2tp, 2tp+1: out psum [128 tok, 2, 256]
                def run(pool):
                    acc = pool.tile([128, CH], F32, tag="acc",
                                    name=f"vacc{jj}_{tp}")
                    accv = acc[:].rearrange("p (a b) -> p a b", b=DV)
                    for tl in (0, 1):
                        t = tp * 2 + tl
                        for d in range(ND):
                            nc.tensor.matmul(
                                accv[:, tl, :],
                                xt_tiles[jj][:, d, t * 128:(t + 1) * 128],
                                wsb["v"][:, d, :],
                                start=(d == 0), stop=(d == ND - 1))
                    nc.vector.tensor_copy(
                        vsbs[jj][:, 8 * tp:8 * tp + 8, 0:DH],
                        acc[:].rearrange("p (u c) -> p u c", c=DH))
                return run

            def oproj_half(jj, tt, half, evac_eng="v"):
                def run(pool):
                    o = pool.tile([128, CH], F32, tag="acc",
                                  name=f"o{jj}_{tt}_{half}")
                    ts = jj * CH + tt * 128
                    for hp in (0, 1):
                        nc.tensor.matmul(
                            o[:], outT[:, hp, ts:ts + 128],
                            wo_sb[:, hp, half * CH:(half + 1) * CH],
                            start=(hp == 0), stop=(hp == 1))
                    osb = obp.tile([128, CH], BF16, tag="osb",
                                   name=f"osb{jj}_{tt}_{half}")
                    if evac_eng == "s":
                        nc.scalar.activation(
                            osb[:], o[:], mybir.ActivationFunctionType.Copy)
                    else:
                        nc.vector.tensor_copy(osb[:], o[:])
                    nc.sync.dma_start(
                        o_d[ts:ts + 128, half * CH:(half + 1) * CH], osb[:])
                return run

            deferred = []

            reserve = [0]

            def pop_deferred(pool, n=1):
                for _ in range(n):
                    if len(deferred) > reserve[0]:
                        deferred.pop(0)(pool)

            def normalize(jj, hp, av, fillp, mid_hook=None):
                tq0 = jj * CH
                su0 = recp.tile([1, CH], F32, tag="su0", name=f"su0{jj}_{hp}")
                su1 = recp.tile([1, CH], F32, tag="su1", name=f"su1{jj}_{hp}")
                outu = recp.tile([128, CH], F32, tag="outu",
                                 name=f"outu{jj}_{hp}")
                nc.vector.tensor_copy(su0[:], av[0][DH:DVA, :])
                nc.vector.tensor_copy(su1[:], av[1][DH:DVA, :])
                dr = drp.tile([2, CH], F32, tag="dr", name=f"dr{jj}_{hp}")
                nc.sync.dma_start(dr[0:1, :], su0[:])
                nc.sync.dma_start(dr[1:2, :], su1[:])
                nc.vector.tensor_copy(outu[0:DH, :], av[0][0:DH, :])
                nc.vector.tensor_copy(outu[DH:128, :], av[1][0:DH, :])
                r8 = recp.tile([128, 8], F32, tag="r8", name=f"r8{jj}_{hp}")
                nc.sync.dma_start(
                    r8[:], dr[:].rearrange("a b -> (a b)").rearrange(
                        "(p j) -> p j", j=8))
                r8r = recp.tile([128, 8], F32, tag="r8r", name=f"r8r{jj}_{hp}")
                nc.vector.reciprocal(r8r[:], r8[:])
                dr2 = drp.tile([2, CH], F32, tag="dr2", name=f"dr2{jj}_{hp}")
                nc.sync.dma_start(
                    dr2[:].rearrange("a b -> (a b)").rearrange(
                        "(p j) -> p j", j=8), r8r[:])
                rbc = recp.tile([128, CH], F32, tag="rbc",
                                name=f"rbc{jj}_{hp}")
                nc.sync.dma_start(rbc[0:DH, :],
                                  dr2[0:1, :].broadcast_to([DH, CH]))
                nc.sync.dma_start(rbc[DH:128, :],
                                  dr2[1:2, :].broadcast_to([DH, CH]))
                nc.vector.tensor_tensor(
                    out=outT[:, hp, tq0:tq0 + CH], in0=outu[:],
                    in1=rbc[:], op=MULT)
                pop_deferred(fillp)
                if mid_hook is not None:
                    mid_hook()

            def attention_chunk(jj, spsp, avp, fillp, mid_hook=None):
                vis = plans[jj]
                if not vis:
                    return
                tq0 = jj * CH
                first_i = vis[0][0]
                last_i = vis[-1][0]
                avs = {}

                hooked = [mid_hook]

                def emit_av(items):
                    done_hps = []
                    for (hp, (i, l0, l1, m0, m1, pat)), p in items:
                        if hp not in avs:
                            avs[hp] = [avp.tile([DVA, CH], F32, tag="av",
                                                name=f"av{h}_{jj}_{hp}")
                                       for h in (0, 1)]
                        for h in (0, 1):
                            u = (i % 4) * NHPC + hp * 2 + h
                            nc.tensor.matmul(
                                avs[hp][h][:, l0:l1],
                                vsbs[i // 4][:, u, :],
                                p[:, h, l0:l1],
                                start=(i == first_i), stop=(i == last_i),
                                skip_group_check=True)
                        if i == last_i:
                            done_hps.append(hp)
                    for hp in done_hps:
                        normalize(jj, hp, avs[hp], fillp, hooked[0])
                        hooked[0] = None

                stream = [(hp, v) for hp in (0, 1) for v in vis]
                pend = None
                pairs = [stream[x:x + 2] for x in range(0, len(stream), 2)]
                for pidx, pair in enumerate(pairs):
                    cur = []
                    for hp, v in pair:
                        i, l0, l1, m0, m1, pat = v
                        ks = slice(i * TK, (i + 1) * TK)
                        sps = spsp.tile([128, 2, CH], F32, tag="sps",
                                        name=f"sps{jj}_{hp}_{i}")
                        for h in (0, 1):
                            hs = slice(h * DH, (h + 1) * DH)
                            nc.tensor.matmul(
                                sps[:, h, l0:l1], kT[hp][hs, ks],
                                qT[hp][hs, tq0 + l0:tq0 + l1],
                                start=True, stop=True)
                        cur.append(((hp, v), sps))
                    if pidx < len(pairs) - 1:
                        pop_deferred(fillp)
                    cur2 = []
                    for hv, sps in cur:
                        hp, (i, l0, l1, m0, m1, pat) = hv
                        p = ppool.tile([128, 2, CH], BF16, tag="p",
                                       name=f"p{jj}_{hp}_{i}")
                        nc.scalar.activation(p[:, :, l0:l1],
                                             sps[:, :, l0:l1], EXP)
                        if pat >= 0:
                            for h in (0, 1):
                                nc.vector.tensor_tensor(
                                    out=p[:, h, m0:m1], in0=p[:, h, m0:m1],
                                    in1=mask_sb[:, pat, 0:m1 - m0], op=MULT)
                        cur2.append((hv, p))
                    if pidx < len(pairs) - 1:
                        pop_deferred(fillp)
                    if pend is not None:
                        emit_av(pend)
                    pend = cur2
                emit_av(pend)

            # ---------------- emission schedule ----------------
            # DMA order: xt chunk0, dvh0 weights, xt chunk1, dvh1 weights —
            # chunk-0 dvh0 projections start as soon as the first tiles land
            # and attn(0)'s chunk-1 proj fillers get their xt early
            prefetch_xt(0)
            for nm, dt_ in (("q", wq_d), ("k", wk_d)):
                nc.sync.dma_start(
                    wsb[nm][:, 0].rearrange("p e c -> p (e c)"),
                    dt_[:, 0:ND * 128])
            nc.sync.dma_start(
                wsb["v"][:].rearrange("p e c -> p (e c)"), wv_d[:])
            prefetch_xt(1)
            for nm, dt_ in (("q", wq_d), ("k", wk_d)):
                nc.sync.dma_start(
                    wsb[nm][:, 1].rearrange("p e c -> p (e c)"),
                    dt_[:, ND * 128:2 * ND * 128])
            with tc.tile_pool(name="proj0_ps", bufs=1, space="PSUM") as pps0:
                acc0 = {}
                for dvh in (0, 1):
                    for nm in ("q", "k"):
                        acc0[(nm, dvh)] = pps0.tile(
                            [128, CH], F32, tag=f"a{nm}{dvh}",
                            name=f"acc0_{nm}{dvh}")
                for dvh in (0, 1):
                    for d in range(ND):
                        for nm in ("q", "k"):
                            nc.tensor.matmul(
                                acc0[(nm, dvh)][:], wsb[nm][:, dvh, d, :],
                                xt_tiles[0][:, d, :],
                                start=(d == 0), stop=(d == ND - 1))
                    for nm in ("q", "k"):
                        evac(0, nm, dvh, acc0[(nm, dvh)])
                    if dvh == 0:
                        vproj_pair(0, 0)(pps0)
                for tp in (1,):
                    vproj_pair(0, tp)(pps0)
            nc.sync.dma_start(wo_sb[:].rearrange("p a b -> p (a b)"), wo_d[:])
            for mi in range(n_pat):
                nc.sync.dma_start(mask_sb[:, mi, :], mask_d[mi])

            pending_oproj = []
            for jj in range(NCH_B):
                if jj + 1 < NCH_B:
                    for dvh in (0, 1):
                        for nm in ("q", "k"):
                            deferred.append(proj_subblock(jj + 1, nm, dvh))
                    for tp in (0, 1):
                        deferred.append(vproj_pair(jj + 1, tp))
                # o-proj of chunk jj-1: queued here (one full chunk after its
                # normalization started) so popped items never wait on the
                # reciprocal bounce and block the in-order PE queue
                deferred.extend(pending_oproj)
                pending_oproj = []
                # xt prefetch for chunk jj+2 fires mid-chunk (after the
                # first normalize) so this bulk transfer queues BEHIND the
                # V-transpose DMAs of chunk jj+1's fillers on the SP queue
                hook = ((lambda c=jj + 2: prefetch_xt(c))
                        if jj + 2 < NCH_B else None)
                # on the last chunk, hold back a few ready o-proj items so
                # the PE has work during the final normalization bounce
                reserve[0] = 4 if jj == NCH_B - 1 else 0
                with tc.tile_pool(name=f"s_ps{jj}", bufs=2,
                                  space="PSUM") as spsp, \
                     tc.tile_pool(name=f"av_ps{jj}", bufs=3,
                                  space="PSUM") as avp, \
                     tc.tile_pool(name=f"fill_ps{jj}", bufs=1,
                                  space="PSUM") as fillp:
                    attention_chunk(jj, spsp, avp, fillp, hook)
                ev = "s" if jj == NCH_B - 1 else "v"
                pending_oproj = [oproj_half(jj, tt, half, ev)
                                 for tt in range(4) for half in (0, 1)]

            reserve[0] = 0
            with tc.tile_pool(name="tail_ps", bufs=4, space="PSUM") as tailp:
                deferred.extend(pending_oproj)
                while deferred:
                    deferred.pop(0)(tailp)

    nc.compile()
    return nc


def kernel(x, Wq, Wk, Wv, Wo, attn_mask):
    import ml_dtypes
    import concourse.bass_utils as _bu

    BF = ml_dtypes.bfloat16
    x = np.asarray(x, dtype=np.float32)
    Wq = np.asarray(Wq, dtype=np.float32)
    Wk = np.asarray(Wk, dtype=np.float32)
    Wv = np.asarray(Wv, dtype=np.float32)
    Wo = np.asarray(Wo, dtype=np.float32)
    mask = np.asarray(attn_mask).astype(bool)

    plans, patterns = _plan_blocks(mask)
    n_pat = len(patterns)
    key_l = []
    for vis in plans:
        key_l.append(len(vis))
        for v in vis:
            key_l.extend(v)
    key = tuple(key_l)
    if key not in _cache:
        _cache[key] = _build(key, n_pat)
    nc = _cache[key]

    mask_arr = (np.stack(patterns).astype(BF) if n_pat
                else np.zeros((1, TK, CH), BF))
    mask_arr = np.ascontiguousarray(mask_arr)

    xts = [np.ascontiguousarray(x[b].reshape(T, D).T.astype(BF))
           for b in range(B)]

    def wlayout(W, rows, scale=1.0):
        Wc = (W[rows, :] * scale).astype(np.float32)  # [256, 1024]
        # -> [128 dpart, 2 dvh, 8 dtile, 128 dvcol] flat [128, 2048]
        m = Wc.T.reshape(ND, 128, 2, 128).transpose(1, 2, 0, 3)
        return np.ascontiguousarray(m.reshape(128, 2 * ND * 128).astype(BF))

    in_maps = []
    for c in range(NCORES):
        b, g = divmod(c, NCORES // B)
        rows = slice(g * DV, (g + 1) * DV)
        wo_dev = Wo[:, rows].T.reshape(2, 128, D).transpose(1, 0, 2)
        in_maps.append({
            "xt": xts[b],
            "wq": wlayout(Wq, rows, 1.0 / np.sqrt(DH)),
            "wk": wlayout(Wk, rows),
            "wv": np.ascontiguousarray(
                Wv[rows, :].astype(np.float32).T
                .reshape(ND, 128, DV).transpose(1, 0, 2)
                .reshape(128, ND * DV).astype(BF)),
            "wo": np.ascontiguousarray(
                wo_dev.reshape(128, 2 * D).astype(BF)),
            "mask": mask_arr,
        })

    res = _bu.run_bass_kernel_spmd(nc, in_maps, core_ids=list(range(NCORES)))
    out = np.zeros((B, T, D), dtype=np.float32)
    for c in range(NCORES):
        b = c // (NCORES // B)
        out[b] += np.asarray(res.results[c]["o"], dtype=np.float32)
    return out
